# revision 1
# baseline (speedup 1.0000x reference)
"""Trainium2 Bass kernel for nn_Aggregation (sparse block-diagonal attention).

Computation (see reference): a single learned query vector attends, per
(sentence, batch), over that sentence's 32 entity slots:
    k/v = entities @ {Wk,Wv}.T + b;  scores = q . k;  attn = softmax_e(scores)
    ctx = sum_e attn * v;            out = ctx @ Wo.T + bo

Two algebraic reductions make this cheap:
 1. The query is one shared vector, so the K projection folds into a tiny
    fused weight computed on host: scores[t, h] = X[t, :] @ M[:, h] + c_h
    with M[c, h] = sum_hd q[h, hd] * Wk[h*64+hd, c].  The K GEMM vanishes.
 2. The attention weights do not depend on the feature dim, so the
    entity-average commutes with the (linear) V projection:
       ctx[(s,b), d] = sum_c Wv[d, c] * Y[h(d), c, (s,b)],
       Y[h, c, (s,b)] = sum_e attn[s,b,h,e] * X[(s,e,b), c].
    Y costs ~0.5 GMAC/core instead of the 8.6 GMAC V projection.

All matmuls run in fp16 (10 mantissa bits; measured end-to-end error vs
the fp32 reference is ~3e-3 of output RMS, ~10x better than bf16).

Sharding: data-parallel over batch, 8 of 64 batch columns per core.
Token order is the natural (sent, entity, batch); the host pre-packs the
activation shard in fp16 twice: natural X (tokens on partitions) and
transposed X^T (contraction dim on partitions), so the device spends its
cycles on matmuls only.

Per 512-token super-tile (2 sents, 16 (s,b) groups):
  scores^T[h, t'] = M^T @ X^T (+ mask via K=1 matmul)        [PE]
  attn = exp(scores + c_h); attn_n = attn / sum_e            [ACT + DVE]
  attn_exp2[t', (sb,h)] = (R^T @ attn_n) * blockmask         [PE + DVE]
  Y^T[c, (sb, h)] += X_nat.T @ attn_exp2  (over 4 subtiles)  [PE]
Epilogue:
  ctx^T[d, sb] = sum_c Wv^T[c, d] Y^T[c, (sb, h(d))]  (per-head matmuls,
  two heads col-packed per PSUM tile) ; += bv; out = ctx^T.T @ Wo^T + bo.

Self-contained: hardcodes all shapes from the problem spec.
"""

import numpy as np

import concourse.bass as bass
import concourse.tile as tile
from concourse import bacc, mybir, bass_utils

# Problem constants (from spec / setup_inputs)
D = 1024
H = 16
HD = D // H
N_SENTS = 32
N_ENTS = 32
SE = N_SENTS * N_ENTS
B = 64
N_CORES = 8
BC = B // N_CORES            # batch columns per core
TOK = N_SENTS * N_ENTS * BC  # tokens per core = 8192
ST_TOK = 512                 # tokens per super-tile (2 sents x 32 e x 8 b)
N_ST = TOK // ST_TOK         # 16 super-tiles
SB = N_SENTS * BC            # (s, b) rows per core = 256

F32 = mybir.dt.float32
F16 = mybir.dt.float16

_NC_CACHE = {}


def _build(use_mask=True):
    key = ("nc", use_mask)
    if key in _NC_CACHE:
        return _NC_CACHE[key]
    nc = bacc.Bacc("TRN2", target_bir_lowering=False, debug=False)

    # X^T, super-tile-major so each per-tile DMA is one contiguous slice:
    # [c-in-chunk(128), st * 4096 + chunk(8) * 512 + t_local]
    XT = nc.dram_tensor("XT", [128, 8 * TOK], F16, kind="ExternalInput").ap()
    # X natural, super-tile-major: [p(128), st * 4096 + j(4) * D + d]
    XN = nc.dram_tensor("XN", [128, TOK // 128 * D], F16,
                        kind="ExternalInput").ap()
    WVT = nc.dram_tensor("WVT", [128, 8 * D], F16, kind="ExternalInput").ap()
    WOT = nc.dram_tensor("WOT", [128, 8 * D], F16, kind="ExternalInput").ap()
    MW = nc.dram_tensor("MW", [128, 8 * H], F16, kind="ExternalInput").ap()
    CH = nc.dram_tensor("CH", [H, 1], F32, kind="ExternalInput").ap()
    BV = nc.dram_tensor("BV", [128, 8], F32, kind="ExternalInput").ap()
    BO = nc.dram_tensor("BO", [1, D], F16, kind="ExternalInput").ap()
    MASKV = nc.dram_tensor("MASKV", [1, TOK], F16, kind="ExternalInput").ap()
    # R4: [h, combo] = [h == hg(combo)*8 + h_lo(combo)],
    #     combo = hg*128 + sbl*8 + h_lo
    R4 = nc.dram_tensor("R4", [H, 256], F16, kind="ExternalInput").ap()
    # block masks per sent-parity: [row r, combo] = [sbl(combo) == jp*8 + r%8]
    BM = nc.dram_tensor("BM", [128, 2 * 256], F16, kind="ExternalInput").ap()
    OUT = nc.dram_tensor("OUT", [SB, D], F32, kind="ExternalOutput").ap()

    with tile.TileContext(nc) as tc:
        with (
            tc.tile_pool(name="wpool", bufs=1) as wpool,
            tc.tile_pool(name="xpool", bufs=3) as xpool,
            tc.tile_pool(name="attnpool", bufs=3) as apool,
            tc.tile_pool(name="ctxpool", bufs=1) as cpool,
            tc.tile_pool(name="psS", bufs=2, space="PSUM") as psS,
            tc.tile_pool(name="psQ", bufs=2, space="PSUM") as psQ,
            tc.tile_pool(name="psY", bufs=2, space="PSUM") as psY,
            tc.tile_pool(name="psC", bufs=2, space="PSUM") as psC,
        ):
            # ---- constants / weights (loaded once; the two big epilogue
            # weights wvt/wot are DMA'd mid-loop so the first super-tiles'
            # activation loads aren't queued behind 4 MB of weights) ----
            wvt = wpool.tile([128, 8 * D], F16)
            wot = wpool.tile([128, 8 * D], F16)
            mw = wpool.tile([128, 8 * H], F16)
            nc.sync.dma_start(mw[:], MW[:])
            ch = wpool.tile([H, 1], F32)
            bv = wpool.tile([128, 8], F32)
            bo = wpool.tile([1, D], F16)
            maskv = wpool.tile([1, TOK], F16)
            r4 = wpool.tile([H, 256], F16)
            bm = wpool.tile([128, 2 * 256], F16)
            ones = wpool.tile([1, 128], F16)
            nc.vector.memset(ones[:], 1.0)

            # Y^T accumulator: [c-in-chunk, chunk(8) * (H * SB) + h * SB + sb]
            yt = cpool.tile([128, 8 * H * SB], F16)

            for st in range(N_ST):
                t0 = st * ST_TOK
                if 2 <= st < 10:
                    q = st - 2
                    nc.sync.dma_start(wvt[:, q * D:(q + 1) * D],
                                      WVT[:, q * D:(q + 1) * D])
                    nc.sync.dma_start(wot[:, q * D:(q + 1) * D],
                                      WOT[:, q * D:(q + 1) * D])
                # ---- load X^T and X natural for this super-tile ----
                xt = xpool.tile([128, 8 * ST_TOK], F16, tag="xt")
                nc.sync.dma_start(
                    xt[:], XT[:, st * 8 * ST_TOK:(st + 1) * 8 * ST_TOK])
                xn = xpool.tile([128, 4 * D], F16, tag="xn")
                nc.sync.dma_start(
                    xn[:], XN[:, st * 4 * D:(st + 1) * 4 * D])
                if st == 0:
                    # small consts AFTER the first activation tiles so they
                    # don't hold up the first scores matmul on the HWDGE path
                    nc.sync.dma_start(ch[:], CH[:])
                    nc.sync.dma_start(r4[:], R4[:])
                    nc.sync.dma_start(bm[:], BM[:])
                    if use_mask:
                        nc.sync.dma_start(maskv[:], MASKV[:])
                    nc.sync.dma_start(bv[:], BV[:])
                    nc.sync.dma_start(bo[:], BO[:])

                # ---- scores^T [16 h, 512 t'] = M^T X^T + mask ----
                ps_s = psS.tile([H, ST_TOK], F32, tag="ps_s")
                for c in range(8):
                    nc.tensor.matmul(
                        ps_s[:],
                        mw[:, c * H:(c + 1) * H],
                        xt[:, c * ST_TOK:(c + 1) * ST_TOK],
                        start=(c == 0), stop=(c == 7 and not use_mask),
                    )
                if use_mask:
                    nc.tensor.matmul(
                        ps_s[:], ones[:, :H],
                        maskv[:, t0:t0 + ST_TOK],
                        start=False, stop=True,
                    )

                # ---- softmax over e (stride BC inside (s, e, b)) ----
                attn = apool.tile([H, ST_TOK], F16, tag="attn")
                nc.scalar.activation(attn[:], ps_s[:],
                                     mybir.ActivationFunctionType.Exp,
                                     bias=ch[:])
                zsum = apool.tile([H, 16], F32, tag="zsum")
                nc.vector.reduce_sum(
                    zsum[:],
                    attn[:].rearrange("p (s e b) -> p s b e", e=N_ENTS, b=BC),
                    axis=mybir.AxisListType.X)
                zrec = apool.tile([H, 16], F32, tag="zrec")
                nc.vector.reciprocal(zrec[:], zsum[:])
                attn_n = apool.tile([H, ST_TOK], F16, tag="attn_n")
                nc.vector.tensor_mul(
                    attn_n[:].rearrange("p (s e b) -> p s e b", e=N_ENTS, b=BC),
                    attn[:].rearrange("p (s e b) -> p s e b", e=N_ENTS, b=BC),
                    zrec[:].rearrange("p (s b) -> p s b", b=BC)[:, :, None, :]
                    .broadcast_to((H, 2, N_ENTS, BC)),
                )

                # ---- attn_exp2[j]: [128 t'-rows, 256 (hg, sbl, h_lo)] ----
                ax2 = apool.tile([128, 4 * 256], F16, tag="ax2")
                for j in range(4):
                    ps_q = psQ.tile([128, 256], F32, tag="ps_q")
                    nc.tensor.matmul(
                        ps_q[:], attn_n[:, j * 128:(j + 1) * 128], r4[:],
                        start=True, stop=True,
                    )
                    nc.vector.tensor_mul(
                        ax2[:, j * 256:(j + 1) * 256], ps_q[:],
                        bm[:, (j // 2) * 256:(j // 2 + 1) * 256],
                    )

                # ---- Y^T += X_nat.T @ attn_exp2, per c-slice ----
                for cs in range(8):
                    ps_y = psY.tile([128, 256], F32, tag="ps_y")
                    for j in range(4):
                        nc.tensor.matmul(
                            ps_y[:],
                            xn[:, j * D + cs * 128: j * D + (cs + 1) * 128],
                            ax2[:, j * 256:(j + 1) * 256],
                            start=(j == 0), stop=(j == 3),
                        )
                    # scatter copy into yt: psum col hg*128 + sbl*8 + h_lo
                    #  -> yt col cs*(H*SB) + (hg*8 + h_lo)*SB + st*16 + sbl
                    ytv = yt[:].rearrange("p (ch sb) -> p ch sb", sb=SB)
                    src = ps_y[:].rearrange("p (hg sbl hl) -> p hg hl sbl",
                                            hg=2, hl=8)
                    dst = ytv[:, cs * H:(cs + 1) * H, st * 16: st * 16 + 16]
                    dst = dst.rearrange("p (hg hl) sbl -> p hg hl sbl", hg=2)
                    nc.scalar.copy(dst, src)

            # ---- ctx^T per head-pair: [128 d, 256 sb] via col-packed MMs ----
            ctxT_bf = cpool.tile([128, 8 * SB], F16)
            for m0 in range(8):
                ps_ctx = psC.tile([128, SB], F32, tag="ps_c")
                for hh in range(2):
                    h = 2 * m0 + hh
                    for c in range(8):
                        nc.tensor.matmul(
                            ps_ctx[hh * 64:(hh + 1) * 64, :],
                            wvt[:, c * D + h * HD: c * D + h * HD + HD],
                            yt[:, c * (H * SB) + h * SB: c * (H * SB) + (h + 1) * SB],
                            start=(c == 0), stop=(c == 7),
                            tile_position=(0, hh * 64),
                        )
                nc.vector.tensor_add(
                    ctxT_bf[:, m0 * SB:(m0 + 1) * SB],
                    ps_ctx[:],
                    bv[:, m0:m0 + 1].broadcast_to((128, SB)),
                )

            # ---- out projection: OUT[sb, f] = ctx^T.T @ WoT + bo ----
            for mt in range(2):
                fin = cpool.tile([128, D], F32, tag="fin")
                for nh in range(2):
                    ps_f = psC.tile([128, 512], F32, tag="ps_c")
                    for c in range(8):
                        nc.tensor.matmul(
                            ps_f[:],
                            ctxT_bf[:, c * SB + mt * 128: c * SB + (mt + 1) * 128],
                            wot[:, c * D + nh * 512: c * D + (nh + 1) * 512],
                            start=(c == 0), stop=False,
                        )
                    nc.tensor.matmul(
                        ps_f[:], ones[:, :128],
                        bo[:, nh * 512:(nh + 1) * 512],
                        start=False, stop=True,
                    )
                    nc.scalar.copy(fin[:, nh * 512:(nh + 1) * 512], ps_f[:])
                nc.sync.dma_start(OUT[mt * 128:(mt + 1) * 128, :], fin[:])

    nc.compile()
    _NC_CACHE[key] = nc
    return nc


def _prep_host(entities, padding_mask, n_sents, query, in_proj_w, in_proj_b,
               out_proj_w, out_proj_b):
    """Host-side prep: shard + layout/dtype packing + weight fusion."""
    assert int(n_sents) == N_SENTS
    f16 = np.float16
    f32 = np.float32

    Wq = in_proj_w[:D]
    Wk = in_proj_w[D:2 * D]
    Wv = in_proj_w[2 * D:]
    bq = in_proj_b[:D]
    bk = in_proj_b[D:2 * D]
    bv = in_proj_b[2 * D:]
    scale = np.float64(1.0) / np.sqrt(np.float64(HD))

    q_vec = ((query.astype(np.float64) @ Wq.T.astype(np.float64)
              + bq.astype(np.float64)) * scale)
    # M[c, h] = sum_hd q_vec[h*HD+hd] * Wk[h*HD+hd, c];  c_h = q_vec_h . bk_h
    M = np.stack(
        [q_vec[h * HD:(h + 1) * HD] @ Wk.astype(np.float64)[h * HD:(h + 1) * HD, :]
         for h in range(H)], axis=1)  # [D, H]
    c_h = np.array(
        [q_vec[h * HD:(h + 1) * HD] @ bk.astype(np.float64)[h * HD:(h + 1) * HD]
         for h in range(H)])

    def pack_kxn(w_t):  # [1024, N] -> [128, 8*N] chunk-major
        n = w_t.shape[1]
        return np.ascontiguousarray(
            w_t.reshape(8, 128, n).transpose(1, 0, 2).reshape(128, 8 * n))

    WVT = pack_kxn(Wv.T.astype(f32)).astype(f16)
    WOT = pack_kxn(out_proj_w.T.astype(f32)).astype(f16)
    MW = pack_kxn(M.astype(f32)).astype(f16)
    CH = c_h.astype(f32).reshape(H, 1)
    BVp = np.ascontiguousarray(bv.astype(f32).reshape(8, 128).T)  # [128, 8]
    BOp = out_proj_b.astype(f32).reshape(1, D).astype(f16)

    # R4[h, combo] = [h == hg*8 + h_lo], combo = hg*128 + sbl*8 + h_lo
    R4p = np.zeros((H, 256), dtype=f16)
    for combo in range(256):
        hg, rem = divmod(combo, 128)
        h_lo = rem % 8
        R4p[hg * 8 + h_lo, combo] = 1.0
    # BM[r, jp*256 + combo] = [sbl(combo) == jp*8 + r%8]
    BMp = np.zeros((128, 2 * 256), dtype=f16)
    for r in range(128):
        for jp in range(2):
            for combo in range(256):
                sbl = (combo % 128) // 8
                if sbl == jp * 8 + r % 8:
                    BMp[r, jp * 256 + combo] = 1.0

    ent16 = entities.astype(f16)  # [SE, B, D]
    maskf = padding_mask.astype(f32) * f32(-30000.0)

    in_maps = []
    for core in range(N_CORES):
        bsl = slice(core * BC, (core + 1) * BC)
        xflat = ent16[:, bsl, :].reshape(TOK, D)
        # X natural, super-tile-major: [p, st * 4096 + j * D + d]
        xn = np.ascontiguousarray(
            xflat.reshape(N_ST, 4, 128, D).transpose(2, 0, 1, 3)
            .reshape(128, N_ST * 4 * D))
        # X^T, super-tile-major: [p=c-in-chunk, st * 4096 + c_chunk * 512 + tl]
        xt = xflat.T.reshape(8, 128, N_ST, ST_TOK)
        xt = np.ascontiguousarray(
            xt.transpose(1, 2, 0, 3).reshape(128, 8 * TOK))
        maskv = np.ascontiguousarray(
            maskf[:, bsl].reshape(1, TOK)).astype(f16)
        in_maps.append({
            "XT": xt, "XN": xn, "WVT": WVT, "WOT": WOT, "MW": MW, "CH": CH,
            "BV": BVp, "BO": BOp, "MASKV": maskv, "R4": R4p, "BM": BMp,
        })
    return in_maps


def kernel(entities, padding_mask, n_sents, query, in_proj_w, in_proj_b,
           out_proj_w, out_proj_b):
    # Accept jax/np arrays alike; host prep must run in numpy (and the
    # q/Wk fold in float64, which jax with x64 disabled would silently
    # downcast).
    entities = np.asarray(entities)
    padding_mask = np.asarray(padding_mask)
    query = np.asarray(query)
    in_proj_w = np.asarray(in_proj_w)
    in_proj_b = np.asarray(in_proj_b)
    out_proj_w = np.asarray(out_proj_w)
    out_proj_b = np.asarray(out_proj_b)
    n_sents = int(n_sents)
    in_maps = _prep_host(entities, padding_mask, n_sents, query, in_proj_w,
                         in_proj_b, out_proj_w, out_proj_b)
    nc = _build(use_mask=bool(np.any(padding_mask)))
    res = None
    last_err = None
    for attempt in range(3):
        try:
            res = bass_utils.run_bass_kernel_spmd(
                nc, in_maps=in_maps, core_ids=list(range(N_CORES)))
            break
        except Exception as e:  # rare transient device wedge; retry
            last_err = e
            import time as _time
            _time.sleep(3)
    if res is None:
        raise last_err
    out = np.empty((N_SENTS, B, D), dtype=np.float32)
    for core in range(N_CORES):
        o = res.results[core]["OUT"].reshape(N_SENTS, BC, D)
        out[:, core * BC:(core + 1) * BC, :] = o
    return out

